# revision 7
# baseline (speedup 1.0000x reference)
"""Multi-head causal attention (B=4, S=2048, D=1024, H=16, Hd=64) on 8 trn2 cores.

Sharding: data-parallel over batch (4) x tensor-parallel over heads (2 groups
of 8 heads). Core c handles batch c//2 and heads 8*(c%2)..8*(c%2)+7:
  - wq/wk/wv column-parallel (each core owns 512 of the 1024 output dims),
  - wo row-parallel (partial outputs summed on host).

Device-side per core:
  phase 1: qT/kT (transposed, [dq,S]) and v (natural, [S,hd]) projections
  phase 2: per head-pair d, q-swath j: scoresT = kT.T-chunk @ qT-swath (row-
           tiled pair of K=64 matmuls), causal additive mask on diagonal
           tiles, exp on ACT (no max subtraction: scores are O(1), exp is
           safe), PV matmul with a ones-column appended to v so the softmax
           denominator falls out of the same matmul, then normalize.
  phase 3: out_partial = attnT.T @ woT  (row-parallel wo)

Host side: shard/transposes, pair-sum of partials, + wo@bv + bo correction
(bk provably cancels in softmax; bv commutes to a constant because softmax
rows sum to 1).

Math note: softmax computed without max-subtraction (scores ~ N(0,1), exp
overflow impossible in fp32); masked entries get -1e30 pre-exp -> exp = 0.
"""
import sys

sys.path.insert(0, "/opt/trn_rl_repo")

import numpy as np

from concourse import bacc, mybir, tile
from concourse.bass_utils import run_bass_kernel_spmd

B, S, D = 4, 2048, 1024
H, HD = 16, 64
HPC = 8        # heads per core
DPC = HPC * HD  # 512 projection dims per core
SW = 512       # q swath width
NSW = S // SW  # 4
NT = S // 128  # 16 token tiles
ND = D // 128  # 8 contraction chunks

# matmul dtype mode: "f32" (exact, 4x slow), "f32r" (full speed, ~tf32ish),
# "bf16" (full speed, least precise, half DMA/SBUF)
MODE = "f32r"

F32 = mybir.dt.float32
EXPF = mybir.ActivationFunctionType.Exp

_NC_CACHE = {}


def _mm_dt(mode):
    import ml_dtypes
    if mode == "bf16":
        return mybir.dt.bfloat16, ml_dtypes.bfloat16
    if mode == "f32r":
        # float32r: fp32 storage, PE reads reduced mantissa at full rate.
        # np-side arrays stay fp32.
        return mybir.dt.float32r, np.float32
    return F32, np.float32


def _build(mode):
    mdt, _ = _mm_dt(mode)

    def mc(ap):
        return ap

    nc = bacc.Bacc("TRN2", target_bir_lowering=False, debug=False, num_devices=8)

    xT_d = nc.dram_tensor("xT", [D, S], mdt, kind="ExternalInput").ap()
    wqT_d = nc.dram_tensor("wqT", [D, DPC], mdt, kind="ExternalInput").ap()
    wkT_d = nc.dram_tensor("wkT", [D, DPC], mdt, kind="ExternalInput").ap()
    wvT_d = nc.dram_tensor("wvT", [D, DPC], mdt, kind="ExternalInput").ap()
    woT_d = nc.dram_tensor("woT", [DPC, D], mdt, kind="ExternalInput").ap()
    bqT_d = nc.dram_tensor("bqT", [128, 4], F32, kind="ExternalInput").ap()
    cm_d = nc.dram_tensor("cm", [128, 128], F32, kind="ExternalInput").ap()
    out_d = nc.dram_tensor("out", [S, D], F32, kind="ExternalOutput").ap()

    # DRAM views with the 128-partition dim innermost-first
    xT_r = xT_d.rearrange("(c p) s -> p c s", p=128)
    wqT_r = wqT_d.rearrange("(c p) n -> p c n", p=128)
    wkT_r = wkT_d.rearrange("(c p) n -> p c n", p=128)
    wvT_r = wvT_d.rearrange("(c p) n -> p c n", p=128)
    woT_r = woT_d.rearrange("(c p) n -> p c n", p=128)

    with tile.TileContext(nc) as tc:
        with tc.tile_pool(name="persist", bufs=1) as pp:
            qT = [pp.tile([128, S], mdt, tag=f"qT{d}", name=f"qT{d}") for d in range(4)]
            kT = [pp.tile([128, S], mdt, tag=f"kT{d}", name=f"kT{d}") for d in range(4)]
            v3 = [pp.tile([128, HPC, HD + 1], mdt, tag=f"v{t}", name=f"v{t}") for t in range(NT)]
            bqT = pp.tile([128, 4], F32, tag="bqT", name="bqT")
            zb = pp.tile([128, 1], F32, tag="zb", name="zb")
            ones8 = pp.tile([128, HPC], F32, tag="ones8", name="ones8")
            nc.sync.dma_start(bqT[:], bqT_d[:])
            nc.vector.memset(zb[:], 0.0)
            nc.vector.memset(ones8[:], 1.0)

            # ---------------- phase 1: projections ----------------
            with (
                tc.tile_pool(name="p1w", bufs=1) as wp,
                tc.tile_pool(name="p1x", bufs=2) as xp,
                tc.tile_pool(name="p1ps", bufs=6, space="PSUM") as psp,
            ):
                wqt = wp.tile([128, ND, DPC], mdt, tag="wqt", name="wqt")
                wkt = wp.tile([128, ND, DPC], mdt, tag="wkt", name="wkt")
                wvt = wp.tile([128, ND, DPC], mdt, tag="wvt", name="wvt")
                nc.sync.dma_start(wqt[:], wqT_r[:])
                nc.sync.dma_start(wkt[:], wkT_r[:])
                nc.sync.dma_start(wvt[:], wvT_r[:])

                for sj in range(NSW):
                    xsw = xp.tile([128, ND, SW], mdt, tag="xsw", name=f"xsw{sj}")
                    nc.sync.dma_start(xsw[:], xT_r[:, :, SW * sj:SW * (sj + 1)])
                    cols = slice(SW * sj, SW * (sj + 1))
                    for dd in range(4):
                        dq = slice(128 * dd, 128 * (dd + 1))
                        psq = psp.tile([128, SW], F32, tag="proj", name=f"psq{sj}_{dd}")
                        for dk in range(ND):
                            nc.tensor.matmul(
                                psq[:], mc(wqt[:, dk, dq]), mc(xsw[:, dk, :]),
                                start=(dk == 0), stop=(dk == ND - 1),
                            )
                        nc.vector.tensor_scalar_add(qT[dd][:, cols], psq[:], bqT[:, dd:dd + 1])
                        psk = psp.tile([128, SW], F32, tag="proj", name=f"psk{sj}_{dd}")
                        for dk in range(ND):
                            nc.tensor.matmul(
                                psk[:], mc(wkt[:, dk, dq]), mc(xsw[:, dk, :]),
                                start=(dk == 0), stop=(dk == ND - 1),
                            )
                        nc.vector.tensor_copy(kT[dd][:, cols], psk[:])
                    for tt in range(4):
                        t = 4 * sj + tt
                        tok = slice(128 * tt, 128 * (tt + 1))
                        psv = psp.tile([128, SW], F32, tag="proj", name=f"psv{t}")
                        for dk in range(ND):
                            nc.tensor.matmul(
                                psv[:], mc(xsw[:, dk, tok]), mc(wvt[:, dk, :]),
                                start=(dk == 0), stop=(dk == ND - 1),
                            )
                        nc.vector.tensor_copy(
                            v3[t][:, :, 0:HD],
                            psv[:].rearrange("p (h e) -> p h e", h=HPC),
                        )
                        nc.vector.tensor_copy(v3[t][:, :, HD:HD + 1].squeeze(), ones8[:])

            # ---------------- phases 2+3 (interleaved) ----------------
            with tc.tile_pool(name="p23", bufs=1) as ap_:
                aoT = [ap_.tile([128, S], mdt, tag=f"aoT{d}", name=f"aoT{d}") for d in range(4)]

                with (
                    tc.tile_pool(name="p2c", bufs=1) as cmp_,
                    tc.tile_pool(name="p3w", bufs=1) as wp3,
                    tc.tile_pool(name="p2e", bufs=6) as ep,
                    tc.tile_pool(name="p2n", bufs=2) as rp,
                    tc.tile_pool(name="p3s", bufs=4) as sp3,
                    tc.tile_pool(name="p2s", bufs=2, space="PSUM") as ps2,
                    tc.tile_pool(name="p2v", bufs=2, space="PSUM") as pvp,
                ):
                    cm = cmp_.tile([128, 128], F32, tag="cm", name="cm")
                    nc.sync.dma_start(cm[:], cm_d[:])
                    wot = wp3.tile([128, 4, D], mdt, tag="wot", name="wot")
                    nc.sync.dma_start(wot[:], woT_r[:])

                    def emit_scores(dd, sj, i):
                        cols = slice(SW * sj, SW * (sj + 1))
                        krows = slice(128 * i, 128 * (i + 1))
                        ps = ps2.tile([128, 2 * SW], F32, tag="sc", name=f"sc{dd}_{sj}_{i}")
                        nc.tensor.matmul(
                            ps[:, 0:SW],
                            mc(kT[dd][0:64, krows]), mc(qT[dd][0:64, cols]),
                        )
                        nc.tensor.matmul(
                            ps[:, SW:2 * SW],
                            mc(kT[dd][64:128, krows]), mc(qT[dd][64:128, cols]),
                        )
                        return ps

                    def emit_tail(dd, sj, i, ps, pv0, pv1, last):
                        h0, h1 = 2 * dd, 2 * dd + 1
                        t = i - 4 * sj
                        c0 = 128 * t if t >= 0 else 0
                        ex = ep.tile([128, 2 * SW], mdt, tag="ex", name=f"ex{dd}_{sj}_{i}")
                        if t >= 0:
                            nc.vector.tensor_add(ps[:, c0:c0 + 128], ps[:, c0:c0 + 128], cm[:])
                            nc.vector.tensor_add(ps[:, SW + c0:SW + c0 + 128], ps[:, SW + c0:SW + c0 + 128], cm[:])
                            nc.scalar.activation(ex[:, c0:SW], ps[:, c0:SW], EXPF, bias=zb[:], scale=0.125)
                            nc.scalar.activation(ex[:, SW + c0:2 * SW], ps[:, SW + c0:2 * SW], EXPF, bias=zb[:], scale=0.125)
                        else:
                            nc.scalar.activation(ex[:], ps[:], EXPF, bias=zb[:], scale=0.125)
                        nc.tensor.matmul(
                            pv0[0:HD + 1, c0:SW], mc(v3[i][:, h0, :]), mc(ex[:, c0:SW]),
                            start=(i == 0), stop=(i == last),
                        )
                        nc.tensor.matmul(
                            pv1[0:HD + 1, c0:SW], mc(v3[i][:, h1, :]), mc(ex[:, SW + c0:2 * SW]),
                            start=(i == 0), stop=(i == last),
                        )

                    def emit_norm(dd, sj, pv, hh):
                        cols = slice(SW * sj, SW * (sj + 1))
                        r_ = rp.tile([1, SW], F32, tag=f"r{hh}", name=f"r{hh}_{dd}_{sj}")
                        nc.vector.reciprocal(r_[0:1, :], pv[HD:HD + 1, :])
                        rb_ = rp.tile([64, SW], F32, tag=f"rb{hh}", name=f"rb{hh}_{dd}_{sj}")
                        nc.gpsimd.partition_broadcast(rb_[0:64, :], r_[0:1, :])
                        if hh == 0:
                            nc.vector.tensor_mul(aoT[dd][0:64, cols], pv[0:64, :], rb_[0:64, :])
                        else:
                            t1 = rp.tile([64, SW], mdt, tag="t1", name=f"t1_{dd}_{sj}")
                            nc.vector.tensor_mul(t1[:], pv[0:64, :], rb_[0:64, :])
                            nc.sync.dma_start(aoT[dd][64:128, cols], t1[0:64, :])

                    def emit_wo(tt, ee, tag):
                        tok = slice(128 * tt, 128 * (tt + 1))
                        ecols = slice(SW * ee, SW * (ee + 1))
                        pso = pvp.tile([128, SW], F32, tag=tag, name=f"o{tt}_{ee}")
                        for dd in range(4):
                            nc.tensor.matmul(
                                pso[:], mc(aoT[dd][:, tok]), mc(wot[:, dd, ecols]),
                                start=(dd == 0), stop=(dd == 3),
                            )
                        st = sp3.tile([128, SW], F32, tag="st", name=f"st{tt}_{ee}")
                        nc.vector.tensor_copy(st[:], pso[:])
                        nc.sync.dma_start(out_d[tok, ecols], st[:])

                    for sj in range(NSW):
                        last = 4 * sj + 3
                        for dd in range(4):
                            pv0 = pvp.tile([128, SW], F32, tag="pv0", name=f"pv0_{dd}_{sj}")
                            pv1 = pvp.tile([128, SW], F32, tag="pv1", name=f"pv1_{dd}_{sj}")
                            pending = emit_scores(dd, sj, 0)
                            for i in range(last + 1):
                                nxt = emit_scores(dd, sj, i + 1) if i < last else None
                                emit_tail(dd, sj, i, pending, pv0, pv1, last)
                                pending = nxt
                            emit_norm(dd, sj, pv0, 0)
                            emit_norm(dd, sj, pv1, 1)
                        # output projection for this swath's tokens (reuses pv slots)
                        for tt in range(4 * sj, 4 * sj + 4):
                            emit_wo(tt, 0, "pv0")
                            emit_wo(tt, 1, "pv1")

    nc.compile()
    return nc


def _get_nc(mode):
    if mode not in _NC_CACHE:
        _NC_CACHE[mode] = _build(mode)
    return _NC_CACHE[mode]


def _causal_mask_tiles():
    # [128,128] additive triangle: within a diagonal 128-block keep iff q >= p
    p = np.arange(128)[:, None]
    q = np.arange(128)[None, :]
    return np.where(q >= p, np.float32(0.0), np.float32(-1e30)).astype(np.float32)


def kernel(x, mask, wq, bq, wk, bk, wv, bv, wo, bo):
    x = np.asarray(x, dtype=np.float32)
    wq = np.asarray(wq, dtype=np.float32)
    bq = np.asarray(bq, dtype=np.float32)
    wk = np.asarray(wk, dtype=np.float32)
    wv = np.asarray(wv, dtype=np.float32)
    bv = np.asarray(bv, dtype=np.float32)
    wo = np.asarray(wo, dtype=np.float32)
    bo = np.asarray(bo, dtype=np.float32)
    # mask is the causal tril (hardcoded in the kernel); bk cancels in softmax

    nc = _get_nc(MODE)
    _, np_dt = _mm_dt(MODE)

    cmask = _causal_mask_tiles()
    in_maps = []
    for c in range(8):
        b, hg = c // 2, c % 2
        rows = slice(DPC * hg, DPC * (hg + 1))
        in_maps.append({
            "xT": np.ascontiguousarray(x[b].T).astype(np_dt),
            "wqT": np.ascontiguousarray(wq[rows].T).astype(np_dt),
            "wkT": np.ascontiguousarray(wk[rows].T).astype(np_dt),
            "wvT": np.ascontiguousarray(wv[rows].T).astype(np_dt),
            "woT": np.ascontiguousarray(wo[:, rows].T).astype(np_dt),
            "bqT": np.ascontiguousarray(bq[rows].reshape(4, 128).T).astype(np.float32),
            "cm": cmask,
        })

    res = run_bass_kernel_spmd(nc, in_maps, list(range(8))).results

    corr = (wo @ bv) + bo  # bv commutes through softmax-normalized attention
    out = np.empty((B, S, D), dtype=np.float32)
    for b in range(B):
        out[b] = res[2 * b]["out"] + res[2 * b + 1]["out"] + corr
    return out


# revision 9
# speedup vs baseline: 1.1695x; 1.1695x over previous
"""Multi-head causal attention (B=4, S=2048, D=1024, H=16, Hd=64) on 8 trn2 cores.

Sharding: data-parallel over batch (4) x tensor-parallel over heads (2 groups
of 8 heads). Core c handles batch c//2 and heads 8*(c%2)..8*(c%2)+7:
  - wq/wk/wv column-parallel (each core owns 512 of the 1024 output dims),
  - wo row-parallel (partial outputs summed on host).

Device-side per core:
  phase 1: qT/kT (transposed, [dq,S]) and v (natural, [S,hd]) projections
  phase 2: per head-pair d, q-swath j: scoresT = kT.T-chunk @ qT-swath (row-
           tiled pair of K=64 matmuls), causal additive mask on diagonal
           tiles, exp on ACT (no max subtraction: scores are O(1), exp is
           safe), PV matmul with a ones-column appended to v so the softmax
           denominator falls out of the same matmul, then normalize.
  phase 3: out_partial = attnT.T @ woT  (row-parallel wo)

Host side: shard/transposes, pair-sum of partials, + wo@bv + bo correction
(bk provably cancels in softmax; bv commutes to a constant because softmax
rows sum to 1).

Math note: softmax computed without max-subtraction (scores ~ N(0,1), exp
overflow impossible in fp32); masked entries get -1e30 pre-exp -> exp = 0.
"""
import sys

sys.path.insert(0, "/opt/trn_rl_repo")

import numpy as np

from concourse import bacc, mybir, tile
from concourse.bass_utils import run_bass_kernel_spmd

B, S, D = 4, 2048, 1024
H, HD = 16, 64
HPC = 8        # heads per core
DPC = HPC * HD  # 512 projection dims per core
SW = 512       # q swath width
NSW = S // SW  # 4
NT = S // 128  # 16 token tiles
ND = D // 128  # 8 contraction chunks

# matmul dtype mode: "f32" (exact, 4x slow), "f32r" (full speed, ~tf32ish),
# "bf16" (full speed, least precise, half DMA/SBUF)
MODE = "f32r"

F32 = mybir.dt.float32
EXPF = mybir.ActivationFunctionType.Exp

_NC_CACHE = {}


def _mm_dt(mode):
    import ml_dtypes
    if mode == "bf16":
        return mybir.dt.bfloat16, ml_dtypes.bfloat16
    if mode == "f32r":
        # float32r: fp32 storage, PE reads reduced mantissa at full rate.
        # np-side arrays stay fp32.
        return mybir.dt.float32r, np.float32
    return F32, np.float32


def _build(mode):
    mdt, _ = _mm_dt(mode)

    def mc(ap):
        return ap

    nc = bacc.Bacc("TRN2", target_bir_lowering=False, debug=False, num_devices=8)

    xT_d = nc.dram_tensor("xT", [D, S], mdt, kind="ExternalInput").ap()
    wqT_d = nc.dram_tensor("wqT", [D, DPC], mdt, kind="ExternalInput").ap()
    wkT_d = nc.dram_tensor("wkT", [D, DPC], mdt, kind="ExternalInput").ap()
    wvT_d = nc.dram_tensor("wvT", [D, DPC], mdt, kind="ExternalInput").ap()
    woT_d = nc.dram_tensor("woT", [DPC, D], mdt, kind="ExternalInput").ap()
    bqT_d = nc.dram_tensor("bqT", [128, 4], F32, kind="ExternalInput").ap()
    cm_d = nc.dram_tensor("cm", [128, 128], F32, kind="ExternalInput").ap()
    out_d = nc.dram_tensor("out", [S, D], F32, kind="ExternalOutput").ap()

    # DRAM views with the 128-partition dim innermost-first
    xT_r = xT_d.rearrange("(c p) s -> p c s", p=128)
    wqT_r = wqT_d.rearrange("(c p) n -> p c n", p=128)
    wkT_r = wkT_d.rearrange("(c p) n -> p c n", p=128)
    wvT_r = wvT_d.rearrange("(c p) n -> p c n", p=128)
    woT_r = woT_d.rearrange("(c p) n -> p c n", p=128)

    with tile.TileContext(nc) as tc:
        with tc.tile_pool(name="persist", bufs=1) as pp:
            qT = [pp.tile([128, S], mdt, tag=f"qT{d}", name=f"qT{d}") for d in range(4)]
            kT = [pp.tile([128, S], mdt, tag=f"kT{d}", name=f"kT{d}") for d in range(4)]
            v3 = [pp.tile([128, HPC, HD + 1], mdt, tag=f"v{t}", name=f"v{t}") for t in range(NT)]
            bqT = pp.tile([128, 4], F32, tag="bqT", name="bqT")
            zb = pp.tile([128, 1], F32, tag="zb", name="zb")
            ones8 = pp.tile([128, HPC], F32, tag="ones8", name="ones8")
            nc.sync.dma_start(bqT[:], bqT_d[:])
            nc.vector.memset(zb[:], 0.0)
            nc.vector.memset(ones8[:], 1.0)

            # ---------------- phase 1: projections ----------------
            with (
                tc.tile_pool(name="p1w", bufs=1) as wp,
                tc.tile_pool(name="p1x", bufs=2) as xp,
                tc.tile_pool(name="p1ps", bufs=6, space="PSUM") as psp,
            ):
                wqt = wp.tile([128, ND, DPC], mdt, tag="wqt", name="wqt")
                wkt = wp.tile([128, ND, DPC], mdt, tag="wkt", name="wkt")
                wvt = wp.tile([128, ND, DPC], mdt, tag="wvt", name="wvt")
                nc.sync.dma_start(wqt[:], wqT_r[:])
                nc.sync.dma_start(wkt[:], wkT_r[:])
                nc.sync.dma_start(wvt[:], wvT_r[:])

                for sj in range(NSW):
                    xsw = xp.tile([128, ND, SW], mdt, tag="xsw", name=f"xsw{sj}")
                    nc.sync.dma_start(xsw[:], xT_r[:, :, SW * sj:SW * (sj + 1)])
                    cols = slice(SW * sj, SW * (sj + 1))
                    for dd in range(4):
                        dq = slice(128 * dd, 128 * (dd + 1))
                        psq = psp.tile([128, SW], F32, tag="proj", name=f"psq{sj}_{dd}")
                        for dk in range(ND):
                            nc.tensor.matmul(
                                psq[:], mc(wqt[:, dk, dq]), mc(xsw[:, dk, :]),
                                start=(dk == 0), stop=(dk == ND - 1),
                            )
                        nc.vector.tensor_scalar_add(qT[dd][:, cols], psq[:], bqT[:, dd:dd + 1])
                        psk = psp.tile([128, SW], F32, tag="proj", name=f"psk{sj}_{dd}")
                        for dk in range(ND):
                            nc.tensor.matmul(
                                psk[:], mc(wkt[:, dk, dq]), mc(xsw[:, dk, :]),
                                start=(dk == 0), stop=(dk == ND - 1),
                            )
                        nc.vector.tensor_copy(kT[dd][:, cols], psk[:])
                    for tt in range(4):
                        t = 4 * sj + tt
                        tok = slice(128 * tt, 128 * (tt + 1))
                        psv = psp.tile([128, SW], F32, tag="proj", name=f"psv{t}")
                        for dk in range(ND):
                            nc.tensor.matmul(
                                psv[:], mc(xsw[:, dk, tok]), mc(wvt[:, dk, :]),
                                start=(dk == 0), stop=(dk == ND - 1),
                            )
                        nc.vector.tensor_copy(
                            v3[t][:, :, 0:HD],
                            psv[:].rearrange("p (h e) -> p h e", h=HPC),
                        )
                        nc.vector.tensor_copy(v3[t][:, :, HD:HD + 1].squeeze(), ones8[:])

            # ---------------- phases 2+3 (interleaved) ----------------
            with tc.tile_pool(name="p23", bufs=1) as ap_:
                aoT = [ap_.tile([128, S], mdt, tag=f"aoT{d}", name=f"aoT{d}") for d in range(4)]

                with (
                    tc.tile_pool(name="p2c", bufs=1) as cmp_,
                    tc.tile_pool(name="p3w", bufs=1) as wp3,
                    tc.tile_pool(name="p2e", bufs=6) as ep,
                    tc.tile_pool(name="p2n", bufs=2) as rp,
                    tc.tile_pool(name="p3s", bufs=4) as sp3,
                    tc.tile_pool(name="p2s", bufs=2, space="PSUM") as ps2,
                    tc.tile_pool(name="p2v", bufs=2, space="PSUM") as pvp,
                ):
                    cm = cmp_.tile([128, 128], F32, tag="cm", name="cm")
                    nc.sync.dma_start(cm[:], cm_d[:])
                    wot = wp3.tile([128, 4, D], mdt, tag="wot", name="wot")
                    nc.sync.dma_start(wot[:], woT_r[:])

                    def emit_scores(dd, sj, i):
                        cols = slice(SW * sj, SW * (sj + 1))
                        krows = slice(128 * i, 128 * (i + 1))
                        ps = ps2.tile([128, 2 * SW], F32, tag="sc", name=f"sc{dd}_{sj}_{i}")
                        nc.tensor.matmul(
                            ps[:, 0:SW],
                            mc(kT[dd][0:64, krows]), mc(qT[dd][0:64, cols]),
                        )
                        nc.tensor.matmul(
                            ps[:, SW:2 * SW],
                            mc(kT[dd][64:128, krows]), mc(qT[dd][64:128, cols]),
                        )
                        return ps

                    def emit_tail(dd, sj, i, ps, pv0, pv1, last):
                        h0, h1 = 2 * dd, 2 * dd + 1
                        t = i - 4 * sj
                        c0 = 128 * t if t >= 0 else 0
                        ex = ep.tile([128, 2 * SW], mdt, tag="ex", name=f"ex{dd}_{sj}_{i}")
                        if t >= 0:
                            nc.vector.tensor_add(ps[:, c0:c0 + 128], ps[:, c0:c0 + 128], cm[:])
                            nc.vector.tensor_add(ps[:, SW + c0:SW + c0 + 128], ps[:, SW + c0:SW + c0 + 128], cm[:])
                            nc.scalar.activation(ex[:, c0:SW], ps[:, c0:SW], EXPF, bias=zb[:], scale=0.125)
                            nc.scalar.activation(ex[:, SW + c0:2 * SW], ps[:, SW + c0:2 * SW], EXPF, bias=zb[:], scale=0.125)
                        else:
                            nc.scalar.activation(ex[:], ps[:], EXPF, bias=zb[:], scale=0.125)
                        nc.tensor.matmul(
                            pv0[0:HD + 1, c0:SW], mc(v3[i][:, h0, :]), mc(ex[:, c0:SW]),
                            start=(i == 0), stop=(i == last),
                        )
                        nc.tensor.matmul(
                            pv1[0:HD + 1, c0:SW], mc(v3[i][:, h1, :]), mc(ex[:, SW + c0:2 * SW]),
                            start=(i == 0), stop=(i == last),
                        )

                    def emit_norm(dd, sj, pv, hh):
                        cols = slice(SW * sj, SW * (sj + 1))
                        s_ = rp.tile([1, SW], F32, tag=f"s{hh}", name=f"s{hh}_{dd}_{sj}")
                        nc.vector.tensor_copy(s_[0:1, :], pv[HD:HD + 1, :])
                        r_ = rp.tile([1, SW], F32, tag=f"r{hh}", name=f"r{hh}_{dd}_{sj}")
                        nc.vector.reciprocal_approx_fast(out=r_[0:1, :], in_=s_[0:1, :])
                        rb_ = rp.tile([64, SW], F32, tag=f"rb{hh}", name=f"rb{hh}_{dd}_{sj}")
                        nc.gpsimd.partition_broadcast(rb_[0:64, :], r_[0:1, :])
                        if hh == 0:
                            nc.vector.tensor_mul(aoT[dd][0:64, cols], pv[0:64, :], rb_[0:64, :])
                        else:
                            t1 = rp.tile([64, SW], mdt, tag="t1", name=f"t1_{dd}_{sj}")
                            nc.vector.tensor_mul(t1[:], pv[0:64, :], rb_[0:64, :])
                            nc.sync.dma_start(aoT[dd][64:128, cols], t1[0:64, :])

                    def emit_wo(tt, ee, tag):
                        tok = slice(128 * tt, 128 * (tt + 1))
                        ecols = slice(SW * ee, SW * (ee + 1))
                        pso = pvp.tile([128, SW], F32, tag=tag, name=f"o{tt}_{ee}")
                        for dd in range(4):
                            nc.tensor.matmul(
                                pso[:], mc(aoT[dd][:, tok]), mc(wot[:, dd, ecols]),
                                start=(dd == 0), stop=(dd == 3),
                            )
                        st = sp3.tile([128, SW], F32, tag="st", name=f"st{tt}_{ee}")
                        nc.vector.tensor_copy(st[:], pso[:])
                        nc.sync.dma_start(out_d[tok, ecols], st[:])

                    def emit_wo_swath(sj):
                        for tt in range(4 * sj, 4 * sj + 4):
                            emit_wo(tt, 0, "pv0")
                            emit_wo(tt, 1, "pv1")

                    for sj in range(NSW):
                        last = 4 * sj + 3
                        for dd in range(4):
                            pv0 = pvp.tile([128, SW], F32, tag="pv0", name=f"pv0_{dd}_{sj}")
                            pv1 = pvp.tile([128, SW], F32, tag="pv1", name=f"pv1_{dd}_{sj}")
                            pending = emit_scores(dd, sj, 0)
                            for i in range(last + 1):
                                nxt = emit_scores(dd, sj, i + 1) if i < last else None
                                emit_tail(dd, sj, i, pending, pv0, pv1, last)
                                pending = nxt
                            emit_norm(dd, sj, pv0, 0)
                            emit_norm(dd, sj, pv1, 1)
                            # wo for the previous swath, pipelined behind this
                            # swath's attention so it never stalls on norms
                            if sj > 0 and dd == 1:
                                emit_wo_swath(sj - 1)
                    emit_wo_swath(NSW - 1)

    nc.compile()
    return nc


def _get_nc(mode):
    if mode not in _NC_CACHE:
        _NC_CACHE[mode] = _build(mode)
    return _NC_CACHE[mode]


def _causal_mask_tiles():
    # [128,128] additive triangle: within a diagonal 128-block keep iff q >= p
    p = np.arange(128)[:, None]
    q = np.arange(128)[None, :]
    return np.where(q >= p, np.float32(0.0), np.float32(-1e30)).astype(np.float32)


def kernel(x, mask, wq, bq, wk, bk, wv, bv, wo, bo):
    x = np.asarray(x, dtype=np.float32)
    wq = np.asarray(wq, dtype=np.float32)
    bq = np.asarray(bq, dtype=np.float32)
    wk = np.asarray(wk, dtype=np.float32)
    wv = np.asarray(wv, dtype=np.float32)
    bv = np.asarray(bv, dtype=np.float32)
    wo = np.asarray(wo, dtype=np.float32)
    bo = np.asarray(bo, dtype=np.float32)
    # mask is the causal tril (hardcoded in the kernel); bk cancels in softmax

    nc = _get_nc(MODE)
    _, np_dt = _mm_dt(MODE)

    cmask = _causal_mask_tiles()
    in_maps = []
    for c in range(8):
        b, hg = c // 2, c % 2
        rows = slice(DPC * hg, DPC * (hg + 1))
        in_maps.append({
            "xT": np.ascontiguousarray(x[b].T).astype(np_dt),
            "wqT": np.ascontiguousarray(wq[rows].T).astype(np_dt),
            "wkT": np.ascontiguousarray(wk[rows].T).astype(np_dt),
            "wvT": np.ascontiguousarray(wv[rows].T).astype(np_dt),
            "woT": np.ascontiguousarray(wo[:, rows].T).astype(np_dt),
            "bqT": np.ascontiguousarray(bq[rows].reshape(4, 128).T).astype(np.float32),
            "cm": cmask,
        })

    res = run_bass_kernel_spmd(nc, in_maps, list(range(8))).results

    corr = (wo @ bv) + bo  # bv commutes through softmax-normalized attention
    out = np.empty((B, S, D), dtype=np.float32)
    for b in range(B):
        out[b] = res[2 * b]["out"] + res[2 * b + 1]["out"] + corr
    return out


# revision 15
# speedup vs baseline: 1.4245x; 1.2180x over previous
"""Multi-head causal attention (B=4, S=2048, D=1024, H=16, Hd=64) on 8 trn2 cores.

Sharding: data-parallel over batch (4) x tensor-parallel over heads (2 groups
of 8 heads). Core c handles batch c//2 and heads 8*(c%2)..8*(c%2)+7:
  - wq/wk/wv column-parallel (each core owns 512 of the 1024 output dims),
  - wo row-parallel (partial outputs summed on host).

Device-side per core:
  phase 1: qT/kT (transposed, [dq,S]) and v (natural, [S,hd]) projections
  phase 2: per head-pair d, q-swath j: scoresT = kT.T-chunk @ qT-swath (row-
           tiled pair of K=64 matmuls), causal additive mask on diagonal
           tiles, exp on ACT (no max subtraction: scores are O(1), exp is
           safe), PV matmul with a ones-column appended to v so the softmax
           denominator falls out of the same matmul, then normalize.
  phase 3: out_partial = attnT.T @ woT  (row-parallel wo)

Host side: shard/transposes, pair-sum of partials, + wo@bv + bo correction
(bk provably cancels in softmax; bv commutes to a constant because softmax
rows sum to 1).

Math note: softmax computed without max-subtraction (scores ~ N(0,1), exp
overflow impossible in fp32); masked entries get -1e30 pre-exp -> exp = 0.
"""
import sys

sys.path.insert(0, "/opt/trn_rl_repo")

import numpy as np

from concourse import bacc, mybir, tile
from concourse.bass_utils import run_bass_kernel_spmd

B, S, D = 4, 2048, 1024
H, HD = 16, 64
HPC = 8        # heads per core
DPC = HPC * HD  # 512 projection dims per core
SW = 512       # q swath width
NSW = S // SW  # 4
NT = S // 128  # 16 token tiles
ND = D // 128  # 8 contraction chunks

# matmul dtype mode: "f32" (exact, 4x slow), "f32r" (full speed, ~tf32ish),
# "bf16" (full speed, least precise, half DMA/SBUF)
MODE = "f32r"

F32 = mybir.dt.float32
EXPF = mybir.ActivationFunctionType.Exp

_NC_CACHE = {}


def _mm_dt(mode):
    import ml_dtypes
    if mode == "bf16":
        return mybir.dt.bfloat16, ml_dtypes.bfloat16
    if mode == "f32r":
        # float32r: fp32 storage, PE reads reduced mantissa at full rate.
        # np-side arrays stay fp32.
        return mybir.dt.float32r, np.float32
    return F32, np.float32


def _build(mode):
    mdt, _ = _mm_dt(mode)
    # PV-stage dtype: bf16 operands (exp weights + v) halve SBUF; their
    # rounding contributes ~3e-4 through the weighted average.
    pdt = mybir.dt.bfloat16 if mode != "f32" else F32

    def mc(ap):
        return ap

    nc = bacc.Bacc("TRN2", target_bir_lowering=False, debug=False, num_devices=8)

    xT_d = nc.dram_tensor("xT", [D, S], mdt, kind="ExternalInput").ap()
    wqT_d = nc.dram_tensor("wqT", [D, DPC], mdt, kind="ExternalInput").ap()
    wkT_d = nc.dram_tensor("wkT", [D, DPC], mdt, kind="ExternalInput").ap()
    wvT_d = nc.dram_tensor("wvT", [D, DPC], mdt, kind="ExternalInput").ap()
    woT_d = nc.dram_tensor("woT", [DPC, D], mdt, kind="ExternalInput").ap()
    bqT_d = nc.dram_tensor("bqT", [128, 4], F32, kind="ExternalInput").ap()
    cm_d = nc.dram_tensor("cm", [128, 128], F32, kind="ExternalInput").ap()
    out_d = nc.dram_tensor("out", [S, D], F32, kind="ExternalOutput").ap()

    xT_r = xT_d.rearrange("(c p) s -> p c s", p=128)
    wqT_r = wqT_d.rearrange("(c p) n -> p c n", p=128)
    wkT_r = wkT_d.rearrange("(c p) n -> p c n", p=128)
    wvT_r = wvT_d.rearrange("(c p) n -> p c n", p=128)
    woT_r = woT_d.rearrange("(c p) n -> p c n", p=128)

    with tile.TileContext(nc) as tc:
        with (
            tc.tile_pool(name="pers", bufs=1) as pp,
            tc.tile_pool(name="qts", bufs=2) as qp,
            tc.tile_pool(name="aots", bufs=2) as aop,
            tc.tile_pool(name="xp", bufs=2) as xp,
            tc.tile_pool(name="exp", bufs=5) as ep,
            tc.tile_pool(name="rp", bufs=1) as rp,
            tc.tile_pool(name="stp", bufs=1) as sp3,
            tc.tile_pool(name="scp", bufs=3, space="PSUM") as ps2,
            tc.tile_pool(name="pvp", bufs=1, space="PSUM") as pvp,
        ):
            kT = [pp.tile([128, S], mdt, tag=f"kT{d}", name=f"kT{d}") for d in range(4)]
            v3 = [pp.tile([128, HPC, HD + 1], pdt, tag=f"v{t}", name=f"v{t}") for t in range(NT)]
            wqt = pp.tile([128, ND, DPC], mdt, tag="wqt", name="wqt")
            wkt = pp.tile([128, ND, DPC], mdt, tag="wkt", name="wkt")
            wvt = pp.tile([128, ND, DPC], mdt, tag="wvt", name="wvt")
            wot = pp.tile([128, 4, D], mdt, tag="wot", name="wot")
            bqT = pp.tile([128, 4], F32, tag="bqT", name="bqT")
            zb = pp.tile([128, 1], F32, tag="zb", name="zb")
            ones8 = pp.tile([128, HPC], F32, tag="ones8", name="ones8")
            cm = pp.tile([128, 128], F32, tag="cm", name="cm")
            nc.sync.dma_start(bqT[:], bqT_d[:])
            nc.sync.dma_start(cm[:], cm_d[:])
            nc.sync.dma_start(wqt[:], wqT_r[:])
            nc.sync.dma_start(wkt[:], wkT_r[:])
            nc.sync.dma_start(wvt[:], wvT_r[:])
            nc.sync.dma_start(wot[:], woT_r[:])
            nc.vector.memset(zb[:], 0.0)
            nc.vector.memset(ones8[:], 1.0)

            qcur = [None] * 4    # per-dd current swath qT tile
            aocur = [None] * 4   # per-dd current swath attnT tile
            xcur = [None]

            def load_x(sj):
                xsw = xp.tile([128, ND, SW], mdt, tag="xsw", name=f"xsw{sj}")
                nc.sync.dma_start(xsw[:], xT_r[:, :, SW * sj:SW * (sj + 1)])
                return xsw

            filler = []  # FIFO of emission closures (each ~2 matmuls of filler)

            def proj_pair_qk(sj, xsw, which, da, db):
                # two projection outputs share one 2-bank psum tile; dk-steps
                # are queued as filler closures woven into attention i-loops
                wt = wqt if which == "q" else wkt
                box = {}

                def step(dk, box=box):
                    if dk == 0:
                        box["ps"] = ps2.tile([128, 2 * SW], F32, tag="sc", name=f"p{which}{sj}_{da}")
                    ps = box["ps"]
                    nc.tensor.matmul(
                        ps[:, 0:SW], mc(wt[:, dk, 128 * da:128 * da + 128]),
                        mc(xsw[:, dk, :]), start=(dk == 0), stop=(dk == ND - 1))
                    nc.tensor.matmul(
                        ps[:, SW:2 * SW], mc(wt[:, dk, 128 * db:128 * db + 128]),
                        mc(xsw[:, dk, :]), start=(dk == 0), stop=(dk == ND - 1))

                def drain(box=box):
                    ps = box["ps"]
                    cols = slice(SW * sj, SW * (sj + 1))
                    if which == "q":
                        for half, dd in ((0, da), (1, db)):
                            qt = qp.tile([128, SW], mdt, tag=f"qT{dd}", name=f"qT{dd}_{sj}")
                            nc.vector.tensor_scalar_add(
                                qt[:], ps[:, SW * half:SW * (half + 1)], bqT[:, dd:dd + 1])
                            qcur[dd] = qt
                    else:
                        for half, dd in ((0, da), (1, db)):
                            nc.vector.tensor_copy(kT[dd][:, cols], ps[:, SW * half:SW * (half + 1)])

                for dk in range(ND):
                    filler.append(lambda dk=dk: step(dk))
                filler.append(drain)

            def proj_pair_v(sj, xsw, ta, tb):
                box = {}

                def step(dk, box=box):
                    if dk == 0:
                        box["ps"] = ps2.tile([128, 2 * SW], F32, tag="sc", name=f"pv{sj}_{ta}")
                    ps = box["ps"]
                    nc.tensor.matmul(
                        ps[:, 0:SW], mc(xsw[:, dk, 128 * ta:128 * ta + 128]),
                        mc(wvt[:, dk, :]), start=(dk == 0), stop=(dk == ND - 1))
                    nc.tensor.matmul(
                        ps[:, SW:2 * SW], mc(xsw[:, dk, 128 * tb:128 * tb + 128]),
                        mc(wvt[:, dk, :]), start=(dk == 0), stop=(dk == ND - 1))

                def drain(box=box):
                    ps = box["ps"]
                    for half, tloc in ((0, ta), (1, tb)):
                        t = 4 * sj + tloc
                        nc.vector.tensor_copy(
                            v3[t][:, :, 0:HD],
                            ps[:, SW * half:SW * (half + 1)].rearrange("p (h e) -> p h e", h=HPC))
                        nc.vector.tensor_copy(v3[t][:, :, HD:HD + 1].squeeze(), ones8[:])

                for dk in range(ND):
                    filler.append(lambda dk=dk: step(dk))
                filler.append(drain)

            def pop_filler(n):
                for _ in range(n):
                    if not filler:
                        return
                    filler.pop(0)()

            def emit_scores(dd, sj, i, qt):
                krows = slice(128 * i, 128 * (i + 1))
                ps = ps2.tile([128, 2 * SW], F32, tag="sc", name=f"sc{dd}_{sj}_{i}")
                nc.tensor.matmul(ps[:, 0:SW], mc(kT[dd][0:64, krows]), mc(qt[0:64, :]))
                nc.tensor.matmul(ps[:, SW:2 * SW], mc(kT[dd][64:128, krows]), mc(qt[64:128, :]))
                return ps

            def emit_tail(dd, sj, i, ps, pv0, pv1, last):
                h0, h1 = 2 * dd, 2 * dd + 1
                t = i - 4 * sj
                c0 = 128 * t if t >= 0 else 0
                ex = ep.tile([128, 2 * SW], pdt, tag="ex", name=f"ex{dd}_{sj}_{i}")
                if t >= 0:
                    psm = ps[:].rearrange("p (g q) -> p g q", g=2)[:, :, c0:c0 + 128]
                    nc.vector.tensor_add(psm, psm, cm[:].unsqueeze(1).broadcast_to((128, 2, 128)))
                    pse = ps[:].rearrange("p (g q) -> p g q", g=2)[:, :, c0:SW]
                    exe = ex[:].rearrange("p (g q) -> p g q", g=2)[:, :, c0:SW]
                    nc.scalar.activation(exe, pse, EXPF, bias=zb[:], scale=0.125)
                else:
                    nc.scalar.activation(ex[:], ps[:], EXPF, bias=zb[:], scale=0.125)
                nc.tensor.matmul(
                    pv0[0:HD + 1, c0:SW], mc(v3[i][:, h0, :]), mc(ex[:, c0:SW]),
                    start=(i == 0), stop=(i == last))
                nc.tensor.matmul(
                    pv1[0:HD + 1, c0:SW], mc(v3[i][:, h1, :]), mc(ex[:, SW + c0:2 * SW]),
                    start=(i == 0), stop=(i == last))

            def emit_norm(dd, sj, pv, hh):
                s_ = rp.tile([1, SW], F32, tag=f"s{hh}", name=f"s{hh}_{dd}_{sj}")
                nc.vector.tensor_copy(s_[0:1, :], pv[HD:HD + 1, :])
                r_ = rp.tile([1, SW], F32, tag=f"r{hh}", name=f"r{hh}_{dd}_{sj}")
                nc.vector.reciprocal_approx_fast(out=r_[0:1, :], in_=s_[0:1, :])
                rb_ = rp.tile([64, SW], F32, tag=f"rb{hh}", name=f"rb{hh}_{dd}_{sj}")
                nc.gpsimd.partition_broadcast(rb_[0:64, :], r_[0:1, :])
                if hh == 0:
                    nc.vector.tensor_mul(aocur[dd][0:64, :], pv[0:64, :], rb_[0:64, :])
                else:
                    t1 = rp.tile([64, SW], mdt, tag="t1", name=f"t1_{dd}_{sj}")
                    nc.vector.tensor_mul(t1[:], pv[0:64, :], rb_[0:64, :])
                    nc.sync.dma_start(aocur[dd][64:128, :], t1[0:64, :])

            def emit_att(dd, sj, qt):
                last = 4 * sj + 3
                pv0 = pvp.tile([128, SW], F32, tag="pv0", name=f"pvh0_{dd}_{sj}")
                pv1 = pvp.tile([128, SW], F32, tag="pv1", name=f"pvh1_{dd}_{sj}")
                ao = aop.tile([128, SW], mdt, tag=f"aoT{dd}", name=f"aoT{dd}_{sj}")
                aocur[dd] = ao
                pending = emit_scores(dd, sj, 0, qt)
                for i in range(last + 1):
                    nxt = emit_scores(dd, sj, i + 1, qt) if i < last else None
                    emit_tail(dd, sj, i, pending, pv0, pv1, last)
                    pop_filler(1)
                    pending = nxt
                emit_norm(dd, sj, pv0, 0)
                emit_norm(dd, sj, pv1, 1)
                return ao

            def emit_wo(sj, ltt, ao_tiles):
                # one token tile, both 512-col halves in one 2-bank psum tile
                tt = 4 * sj + ltt
                tok = slice(128 * ltt, 128 * (ltt + 1))
                ps = ps2.tile([128, 2 * SW], F32, tag="sc", name=f"o{tt}")
                for ee in range(2):
                    for dd in range(4):
                        nc.tensor.matmul(
                            ps[:, SW * ee:SW * (ee + 1)],
                            mc(ao_tiles[dd][:, tok]), mc(wot[:, dd, SW * ee:SW * (ee + 1)]),
                            start=(dd == 0), stop=(dd == 3))
                st = sp3.tile([128, 2 * SW], F32, tag="st", name=f"st{tt}")
                nc.vector.tensor_copy(st[:], ps[:])
                nc.sync.dma_start(out_d[128 * tt:128 * (tt + 1), :], st[:])

            # ---------------- weave ----------------
            xcur[0] = load_x(0)
            for which, da, db in (("q", 0, 1), ("q", 2, 3), ("k", 0, 1),
                                  ("k", 2, 3)):
                proj_pair_qk(0, xcur[0], which, da, db)
            proj_pair_v(0, xcur[0], 0, 1)
            proj_pair_v(0, xcur[0], 2, 3)
            pop_filler(len(filler))  # prologue: emit all of swath 0's projections

            ao_prev = None
            for sj in range(NSW):
                xnxt = load_x(sj + 1) if sj + 1 < NSW else None
                if xnxt is not None:
                    # queue next swath's projections; they emit as filler
                    proj_pair_qk(sj + 1, xnxt, "q", 0, 1)
                    proj_pair_qk(sj + 1, xnxt, "q", 2, 3)
                    proj_pair_qk(sj + 1, xnxt, "k", 0, 1)
                    proj_pair_qk(sj + 1, xnxt, "k", 2, 3)
                    proj_pair_v(sj + 1, xnxt, 0, 1)
                    proj_pair_v(sj + 1, xnxt, 2, 3)
                ao_now = [None] * 4
                qnow = list(qcur)  # this swath's q tiles (proj(sj+1) replaces qcur)
                for dd in range(4):
                    ao_now[dd] = emit_att(dd, sj, qnow[dd])
                    # drain some filler between blocks, plus wo for sj-1
                    pop_filler(3 if sj > 0 else 9)
                    if ao_prev is not None:
                        emit_wo(sj - 1, dd, ao_prev)
                ao_prev = ao_now
                if xnxt is not None:
                    xcur[0] = xnxt
                pop_filler(len(filler))  # flush: qcur must be current before next swath
            for ltt in range(4):
                emit_wo(NSW - 1, ltt, ao_prev)

    nc.compile()
    return nc


def _get_nc(mode):
    if mode not in _NC_CACHE:
        _NC_CACHE[mode] = _build(mode)
    return _NC_CACHE[mode]


def _causal_mask_tiles():
    # [128,128] additive triangle: within a diagonal 128-block keep iff q >= p
    p = np.arange(128)[:, None]
    q = np.arange(128)[None, :]
    return np.where(q >= p, np.float32(0.0), np.float32(-1e30)).astype(np.float32)


def kernel(x, mask, wq, bq, wk, bk, wv, bv, wo, bo):
    x = np.asarray(x, dtype=np.float32)
    wq = np.asarray(wq, dtype=np.float32)
    bq = np.asarray(bq, dtype=np.float32)
    wk = np.asarray(wk, dtype=np.float32)
    wv = np.asarray(wv, dtype=np.float32)
    bv = np.asarray(bv, dtype=np.float32)
    wo = np.asarray(wo, dtype=np.float32)
    bo = np.asarray(bo, dtype=np.float32)
    # mask is the causal tril (hardcoded in the kernel); bk cancels in softmax

    nc = _get_nc(MODE)
    _, np_dt = _mm_dt(MODE)

    cmask = _causal_mask_tiles()
    in_maps = []
    for c in range(8):
        b, hg = c // 2, c % 2
        rows = slice(DPC * hg, DPC * (hg + 1))
        in_maps.append({
            "xT": np.ascontiguousarray(x[b].T).astype(np_dt),
            "wqT": np.ascontiguousarray(wq[rows].T).astype(np_dt),
            "wkT": np.ascontiguousarray(wk[rows].T).astype(np_dt),
            "wvT": np.ascontiguousarray(wv[rows].T).astype(np_dt),
            "woT": np.ascontiguousarray(wo[:, rows].T).astype(np_dt),
            "bqT": np.ascontiguousarray(bq[rows].reshape(4, 128).T).astype(np.float32),
            "cm": cmask,
        })

    res = run_bass_kernel_spmd(nc, in_maps, list(range(8))).results

    corr = (wo @ bv) + bo  # bv commutes through softmax-normalized attention
    out = np.empty((B, S, D), dtype=np.float32)
    for b in range(B):
        out[b] = res[2 * b]["out"] + res[2 * b + 1]["out"] + corr
    return out
